# revision 40
# baseline (speedup 1.0000x reference)
"""BiDAF forward kernel for Trainium2, data-parallel over batch on 8 NeuronCores.

Layout strategy (per core, Bc=4 batch elements):
  - Everything kept "transposed": features on SBUF partitions, time t on the
    free dimension. This makes the attention softmax a free-dim softmax, the
    input projections plain matmuls, and the GRU gate math per-partition.
  - GRU scans (the critical path: 3 layers x 512 sequential steps, fully
    unrolled, fwd/bwd chains independent so their serial latencies overlap):
      * gate pre-activations accumulate in PSUM stripes; gx_rz is injected
        into the accumulation by identity matmuls (no DVE add needed);
      * h_t = zh_t - t2_t with zh = z*h_prev, t2 = (z-1)*n, and
        Whh@h_t = Whh@zh + (-Whh)@t2 accumulated from the two pieces as they
        are produced, so next-step matmuls don't wait for the final h_t add;
      * fused scalar_tensor_tensor ops handle the bhh_n bias and the (z-1)
        scaling; per step per dir: 5 DVE + 2 ACT + 8 PE matmuls.
  - m tiles are float32r so boundary projections run 1-cycle/row matmuls.
  - All biases except bhh_n are folded into the precomputed input projections.
"""

import os
import sys

for _p in ("/opt/trn_rl_repo", "/root/.axon_site/_ro/trn_rl_repo"):
    if os.path.isdir(_p) and _p not in sys.path:
        sys.path.insert(0, _p)

import numpy as np

import concourse.bacc as bacc
import concourse.bass as bass
import concourse.tile as tile
from concourse import masks, mybir
from concourse.alu_op_type import AluOpType
from concourse.bass_utils import run_bass_kernel_spmd

F32 = mybir.dt.float32
F32R = mybir.dt.float32r
AF = mybir.ActivationFunctionType
AX = mybir.AxisListType

N_CORES = 8
B_FULL = 32
BC = B_FULL // N_CORES  # 4
T_FULL = 512
J = 64
D2 = 200
H = 100

_prog_cache = {}


def _r32(ap):
    """View an fp32 AP as float32r for fast (1 cyc/row, N>=256) matmuls."""
    return ap.bitcast(F32R)


def build_program(T=T_FULL, use_f32r=True, unroll=16, loop=True):
    nc = bacc.Bacc("TRN2", target_bir_lowering=False, debug=False,
                   num_devices=N_CORES)

    # ---- DRAM I/O ----------------------------------------------------------
    c_dram = nc.dram_tensor("c", [BC, T, D2], F32, kind="ExternalInput").ap()
    q_dram = nc.dram_tensor("q", [BC, J, D2], F32, kind="ExternalInput").ap()
    # packed params (see host prep below)
    whhT_dram = nc.dram_tensor("whhT", [H, 1800], F32, kind="ExternalInput").ap()
    whhTn_dram = nc.dram_tensor("whhTn", [H, 1800], F32, kind="ExternalInput").ap()
    bhn_dram = nc.dram_tensor("bhn", [H, 6], F32, kind="ExternalInput").ap()
    gxb_dram = nc.dram_tensor("gxb", [H, 18], F32, kind="ExternalInput").ap()
    wih0_dram = nc.dram_tensor("wih0T", [800, 600], F32, kind="ExternalInput").ap()
    wih1_dram = nc.dram_tensor("wih1T", [D2, 600], F32, kind="ExternalInput").ap()
    wih2_dram = nc.dram_tensor("wih2T", [D2, 600], F32, kind="ExternalInput").ap()
    wsT_dram = nc.dram_tensor("wsT", [H, 6], F32, kind="ExternalInput").ap()
    wpT_dram = nc.dram_tensor("wpT", [H, 20], F32, kind="ExternalInput").ap()
    ps_dram = nc.dram_tensor("p_start", [BC, T], F32, kind="ExternalOutput").ap()
    pe_dram = nc.dram_tensor("p_end", [BC, T], F32, kind="ExternalOutput").ap()

    TK = T // 128  # number of 128-row t-chunks

    DTR = F32R if use_f32r else F32

    with tile.TileContext(nc) as tc:
        from contextlib import ExitStack
        ctx = ExitStack()
        with ctx:
            consts = ctx.enter_context(tc.tile_pool(name="consts", bufs=1))
            gxpool = ctx.enter_context(tc.tile_pool(name="gx", bufs=1))
            mpool = ctx.enter_context(tc.tile_pool(name="m", bufs=1))

            # ---- constants / weights ---------------------------------------
            ident = consts.tile([128, 128], F32)
            masks.make_identity(nc, ident[:])
            ones64 = consts.tile([1, J], F32)
            nc.vector.memset(ones64[:], 1.0)

            whhT = consts.tile([H, 1800], F32)
            nc.sync.dma_start(out=whhT[:], in_=whhT_dram[:])
            whhTn = consts.tile([H, 1800], F32, name="whhTn")
            nc.sync.dma_start(out=whhTn[:], in_=whhTn_dram[:])
            bhn_t = consts.tile([H, 6], F32)
            nc.sync.dma_start(out=bhn_t[:], in_=bhn_dram[:])
            gxb = consts.tile([H, 18], F32)
            nc.sync.dma_start(out=gxb[:], in_=gxb_dram[:])
            wsT = consts.tile([H, 6], F32)
            nc.sync.dma_start(out=wsT[:], in_=wsT_dram[:])
            wpT = consts.tile([H, 20], F32)
            nc.sync.dma_start(out=wpT[:], in_=wpT_dram[:])
            wih1 = [consts.tile([H, 600], F32R, tag=f"wih1_{k}", name=f"wih1_{k}") for k in range(2)]
            wih2 = [consts.tile([H, 600], F32R, tag=f"wih2_{k}", name=f"wih2_{k}") for k in range(2)]
            for k in range(2):
                wst1 = consts.tile([H, 600], F32, tag=f"wst1_{k}", name=f"wst1_{k}")
                nc.sync.dma_start(out=wst1[:], in_=wih1_dram[100 * k:100 * k + 100, :])
                nc.vector.tensor_copy(wih1[k][:], wst1[:])
                wst2 = consts.tile([H, 600], F32, tag=f"wst2_{k}", name=f"wst2_{k}")
                nc.sync.dma_start(out=wst2[:], in_=wih2_dram[100 * k:100 * k + 100, :])
                nc.vector.tensor_copy(wih2[k][:], wst2[:])

            # h0 = zeros
            h0 = consts.tile([H, 2 * BC], F32)
            nc.vector.memset(h0[:], 0.0)

            # gx buffers (reused across the 3 layers)
            # rz: [t][rrrr zzzz] interleave-8 ; n: [t][nnnn]
            gx_rz = [gxpool.tile([H, 8 * T], F32, tag=f"gxrz{d}", name=f"gxrz{d}") for d in range(2)]
            gx_n = [gxpool.tile([H, 4 * T], F32, tag=f"gxn{d}", name=f"gxn{d}") for d in range(2)]

            # m buffers: scan outputs, [t][bbbb] interleave-4. float32r so the
            # boundary projections can run 1-cycle/row matmuls (h' writes round)
            m1 = [mpool.tile([H, 4 * T], F32R, tag=f"m1{d}", name=f"m1{d}") for d in range(2)]
            m2 = [mpool.tile([H, 4 * T], F32R, tag=f"m2{d}", name=f"m2{d}") for d in range(2)]
            m3 = [mpool.tile([H, 4 * T], F32R, tag=f"m3{d}", name=f"m3{d}") for d in range(2)]

            # head logits g-part, [b][t] blocks (bounced through DRAM to save SBUF)
            lgS_dram = nc.dram_tensor("lgS_scratch", [BC, T], F32).ap()
            lgE_dram = nc.dram_tensor("lgE_scratch", [BC, T], F32).ap()

            # ---------------------------------------------------------------
            # Stage 1+2 per batch element: attention, g^T features, gx0, head
            # g-part logits.
            # ---------------------------------------------------------------
            with tc.tile_pool(name="wih0", bufs=1) as wih0p, \
                 tc.tile_pool(name="stg", bufs=1) as stg, \
                 tc.tile_pool(name="feat", bufs=1) as feat, \
                 tc.tile_pool(name="spsum", bufs=2, space=bass.MemorySpace.PSUM) as spsum, \
                 tc.tile_pool(name="spsum1", bufs=3, space=bass.MemorySpace.PSUM) as spsum1, \
                 tc.tile_pool(name="simpool", bufs=1, space=bass.MemorySpace.PSUM) as simpool, \
                 tc.tile_pool(name="gxpsum", bufs=2, space=bass.MemorySpace.PSUM) as gxpsum:

                wih0 = [wih0p.tile([H, 600], DTR, tag=f"wih0_{k}", name=f"wih0_{k}") for k in range(8)]
                wsTr = wih0p.tile([H, 6], DTR, name="wsTr")
                nc.vector.tensor_copy(wsTr[:], wsT[:])
                wpTr = wih0p.tile([H, 20], DTR, name="wpTr")
                nc.vector.tensor_copy(wpTr[:], wpT[:])
                for k in range(8):
                    wst = stg.tile([H, 600], F32, tag="wst", name="wst")
                    nc.sync.dma_start(out=wst[:], in_=wih0_dram[100 * k:100 * k + 100, :])
                    nc.vector.tensor_copy(wih0[k][:], wst[:])

                for b in range(BC):
                    # -- load & transpose c, q --
                    c_nat = [stg.tile([128, D2], F32, tag=f"cnat{k}", name=f"cnat{k}") for k in range(TK)]
                    for k in range(TK):
                        nc.sync.dma_start(out=c_nat[k][:],
                                          in_=c_dram[b, 128 * k:128 * k + 128, :])
                    q_nat = stg.tile([J, D2], F32, tag="qnat")
                    nc.sync.dma_start(out=q_nat[:], in_=q_dram[b, :, :])

                    cT = [feat.tile([H, T], DTR, tag=f"cT{dc}", name=f"cT{dc}") for dc in range(2)]
                    uT = [feat.tile([H, T], DTR, tag=f"uT{dc}", name=f"uT{dc}") for dc in range(2)]
                    cuT = [feat.tile([H, T], DTR, tag=f"cuT{dc}", name=f"cuT{dc}") for dc in range(2)]
                    chT = [feat.tile([H, T], DTR, tag=f"chT{dc}", name=f"chT{dc}") for dc in range(2)]
                    qT = [stg.tile([H, J], DTR, tag=f"qT{dc}", name=f"qT{dc}") for dc in range(2)]

                    for dc in range(2):
                        for k in range(TK):
                            ptr = spsum.tile([H, 128], F32, tag="tr", name="ptr")
                            nc.tensor.transpose(ptr[:], c_nat[k][:, 100 * dc:100 * dc + 100],
                                                ident[:, 0:128])
                            nc.vector.tensor_copy(cT[dc][:, 128 * k:128 * k + 128], ptr[:])
                        pq = spsum.tile([H, J], F32, tag="tr", name="pq")
                        nc.tensor.transpose(pq[:], q_nat[:, 100 * dc:100 * dc + 100],
                                            ident[0:J, 0:J])
                        nc.vector.tensor_copy(qT[dc][:], pq[:])

                    # -- sim^T = (q w_hu) @ c^T + broadcast terms --
                    cwT = [stg.tile([H, T], DTR, tag=f"cwT{dc}", name=f"cwT{dc}") for dc in range(2)]
                    for dc in range(2):
                        nc.vector.tensor_scalar_mul(cwT[dc][:], cT[dc][:],
                                                    wsT[:, 4 + dc:5 + dc])
                    # w_h . c  -> (1, T)
                    wc_ps = spsum1.tile([1, T], F32, tag="small", name="wc")
                    for dc in range(2):
                        nc.tensor.matmul(wc_ps[:], wsT[:, dc:dc + 1],
                                         cT[dc][:].bitcast(F32),
                                         start=(dc == 0), stop=(dc == 1))
                    wc_s = stg.tile([1, T], F32, tag="wc_s")
                    nc.vector.tensor_copy(wc_s[:], wc_ps[:])
                    # w_u . q -> (J, 1)
                    wuq_ps = spsum1.tile([J, 1], F32, tag="small", name="wuq")
                    for dc in range(2):
                        nc.tensor.matmul(wuq_ps[:], qT[dc][:].bitcast(F32),
                                         wsT[:, 2 + dc:3 + dc],
                                         start=(dc == 0), stop=(dc == 1))
                    wuq_s = stg.tile([J, 1], F32, tag="wuq_s")
                    nc.vector.tensor_copy(wuq_s[:], wuq_ps[:])

                    simT = simpool.tile([J, T], F32, tag="simT", name="simT")
                    nc.tensor.matmul(simT[:], (qT[0][:]), (cwT[0][:]),
                                     start=True, stop=False)
                    nc.tensor.matmul(simT[:], (qT[1][:]), (cwT[1][:]),
                                     start=False, stop=False)
                    nc.tensor.matmul(simT[:], ones64[:], wc_s[:],
                                     start=False, stop=True)

                    # -- attn_a = softmax over t (free dim); w_u.q term drops --
                    negmax = stg.tile([J, 1], F32, tag="negmax")
                    nc.vector.tensor_reduce(negmax[:], simT[:], AX.X, AluOpType.max,
                                            negate=True)
                    attnT = stg.tile([J, T], DTR, tag="attnT")
                    sums = stg.tile([J, 1], F32, tag="sums")
                    nc.scalar.activation(attnT[:], simT[:], AF.Exp, bias=negmax[:],
                                         accum_out=sums[:])
                    rsum = stg.tile([J, 1], F32, tag="rsum")
                    nc.vector.reciprocal(rsum[:], sums[:])
                    qs = stg.tile([J, D2], DTR, tag="qs")
                    nc.vector.tensor_scalar_mul(qs[:], q_nat[:], rsum[:])

                    # -- u_tilde^T = (q_scaled)^T @ attn^T --
                    for dc in range(2):
                        up = spsum.tile([H, T], F32, tag="tr", name="up")
                        nc.tensor.matmul(up[:], (qs[:, 100 * dc:100 * dc + 100]),
                                         (attnT[:]), start=True, stop=True)
                        nc.vector.tensor_copy(uT[dc][:], up[:])
                        nc.vector.tensor_mul(cuT[dc][:], cT[dc][:], uT[dc][:])

                    # -- attn_b path: needs w_u.q term --
                    simTb = stg.tile([J, T], F32, tag="simTb")
                    nc.vector.tensor_scalar_add(simTb[:], simT[:], wuq_s[:])
                    mxj = stg.tile([1, T], F32, tag="mxj")
                    nc.gpsimd.tensor_reduce(mxj[:], simTb[:], AX.C, AluOpType.max)
                    negmax2 = stg.tile([1, 1], F32, tag="negmax2")
                    nc.vector.tensor_reduce(negmax2[:], mxj[:], AX.X, AluOpType.max,
                                            negate=True)
                    eb = stg.tile([1, T], F32, tag="eb")
                    sb = stg.tile([1, 1], F32, tag="sb")
                    nc.scalar.activation(eb[:], mxj[:], AF.Exp, bias=negmax2[:],
                                         accum_out=sb[:])
                    rb = stg.tile([1, 1], F32, tag="rb")
                    nc.vector.reciprocal(rb[:], sb[:])
                    attnb = stg.tile([1, T], F32, tag="attnb")
                    nc.vector.tensor_scalar_mul(attnb[:], eb[:], rb[:])
                    # transpose attn_b to (t, 1) chunks
                    abT = stg.tile([128, TK], F32, tag="abT")
                    for k in range(TK):
                        pab = spsum1.tile([128, 1], F32, tag="small", name="pab")
                        nc.tensor.transpose(pab[:], attnb[:, 128 * k:128 * k + 128],
                                            ident[0:1, 0:1])
                        nc.vector.tensor_copy(abT[:, k:k + 1], pab[:])
                    # h_tilde (100, 1) per d-chunk
                    htS = stg.tile([H, 2], F32, tag="htS")
                    for dc in range(2):
                        htp = spsum1.tile([H, 1], F32, tag="small", name="htp")
                        for k in range(TK):
                            nc.tensor.matmul(htp[:], c_nat[k][:, 100 * dc:100 * dc + 100],
                                             abT[:, k:k + 1], start=(k == 0),
                                             stop=(k == TK - 1))
                        nc.vector.tensor_copy(htS[:, dc:dc + 1], htp[:])
                    for dc in range(2):
                        nc.vector.tensor_scalar_mul(chT[dc][:], cT[dc][:],
                                                    htS[:, dc:dc + 1])

                    # -- gx0 projection: 6 gate-chunks x 8 K-blocks --
                    rhs_blocks = [cT[0], cT[1], uT[0], uT[1], cuT[0], cuT[1],
                                  chT[0], chT[1]]
                    gx_rz3 = [gx_rz[d][:].rearrange("p (t k) -> p t k", k=8)
                              for d in range(2)]
                    gx_n3 = [gx_n[d][:].rearrange("p (t k) -> p t k", k=4)
                             for d in range(2)]
                    for g in range(6):
                        d, gate = divmod(g, 3)
                        pg = gxpsum.tile([H, T], F32, tag="pg")
                        for kb in range(8):
                            nc.tensor.matmul(pg[:],
                                             (wih0[kb][:, 100 * g:100 * g + 100]),
                                             (rhs_blocks[kb][:]),
                                             start=(kb == 0), stop=(kb == 7))
                        bias_col = gxb[:, g:g + 1]
                        if gate < 2:
                            out_ap = gx_rz3[d][:, :, b + 4 * gate]
                        else:
                            out_ap = gx_n3[d][:, :, b]
                        nc.vector.tensor_scalar_add(out_ap, pg[:], bias_col)

                    # -- head logits, g-part --
                    for head, lgd in ((0, lgS_dram), (1, lgE_dram)):
                        lp = spsum1.tile([1, T], F32, tag="small", name="lp")
                        for kb in range(8):
                            nc.tensor.matmul(lp[:],
                                             wpTr[:, 10 * head + kb:10 * head + kb + 1],
                                             rhs_blocks[kb][:],
                                             start=(kb == 0), stop=(kb == 7))
                        lgs = stg.tile([1, T], F32, tag="lgs", name="lgs")
                        nc.vector.tensor_copy(lgs[:], lp[:])
                        nc.sync.dma_start(out=lgd[b:b + 1, :], in_=lgs[:])

            # ---------------------------------------------------------------
            # Stage 3: the three bidirectional GRU scans.
            #
            # Per step per dir:
            #   PE : r,z matmuls accumulate (start=False) onto a PSUM stripe
            #        pre-filled with gx_rz (kills the DVE arz add);
            #        n matmul into its own PSUM stripe (start=True).
            #   ACT: rz = sigmoid(psum_rz)         [H,8]
            #   DVE: u  = (ps_n + bhh_n) * r       (scalar_tensor_tensor)
            #        v  = u + gx_n
            #   ACT: nt = tanh(v)
            #   DVE: zh = z * h_prev               (off critical path)
            #        t2 = (z - 1) * nt             (h' = zh - t2)
            #        h' = zh - t2
            # ---------------------------------------------------------------
            SRZ = 64   # steps per rz PSUM stripe (8 cols/step -> 512 f32, 1 bank)
            SN = 128   # steps per n PSUM stripe (4 cols/step)

            def scan_layer(lidx, mout):
                """GRU bi-scan with matmul splitting.

                h_t = zh_t - t2_t with zh_t = z*h_{t-1}, t2_t = (z-1)*n_t, so
                Whh@h_t = Whh@zh_t + (-Whh)@t2_t is accumulated in PSUM from
                the two pieces as they become ready -- the gate matmuls for
                step t+1 no longer wait for the final h_t add. gx_rz is
                injected into the PSUM accumulation via identity matmuls.
                The two dirs stay fully independent so their serial chains
                overlap on the engines.
                """
                wbase = lidx * 600  # per layer: 2 dirs x 3 gates x 100

                with tc.tile_pool(name=f"scan{lidx}", bufs=12) as sp, \
                     tc.tile_pool(name=f"rzps{lidx}", bufs=2, space=bass.MemorySpace.PSUM) as rzpool, \
                     tc.tile_pool(name=f"nps{lidx}", bufs=2, space=bass.MemorySpace.PSUM) as npool:

                    cur = {}  # (kind, d) -> current stripe tile

                    def region(d, s):
                        """PSUM slices for scan position s of dir d (alloc on
                        stripe boundary, memoized)."""
                        if cur.get(("rzi", d)) != s // SRZ:
                            cur[("rz", d)] = rzpool.tile([H, 8 * SRZ], F32,
                                                         tag=f"rz{d}", name=f"rzp{d}")
                            cur[("rzi", d)] = s // SRZ
                        if cur.get(("ni", d)) != s // SN:
                            cur[("n", d)] = npool.tile([H, 4 * SN], F32,
                                                       tag=f"n{d}", name=f"np{d}")
                            cur[("ni", d)] = s // SN
                        if d == 0:
                            ro, no = 8 * (s % SRZ), 4 * (s % SN)
                        else:
                            ro = 8 * (SRZ - 1 - (s % SRZ))
                            no = 4 * (SN - 1 - (s % SN))
                        return cur[("rz", d)], ro, cur[("n", d)], no

                    def produce(d, s, i, zh_ap, t2_ap):
                        """Accumulate gate pre-activations for scan position s
                        (time index i) of dir d: gx inject + Whh@zh (+ -Whh@t2)."""
                        rzp, ro, npx, no = region(d, s)
                        wofs = wbase + d * 300
                        # one accumulation group may be open per PSUM bank at a
                        # time, so finish each gate region before the next
                        for g in range(3):
                            out = (npx[:, no:no + 4] if g == 2
                                   else rzp[:, ro + 4 * g:ro + 4 * g + 4])
                            w = whhT[:, wofs + 100 * g:wofs + 100 * g + 100]
                            wn = whhTn[:, wofs + 100 * g:wofs + 100 * g + 100]
                            if g < 2:
                                nc.tensor.matmul(out, ident[0:H, 0:H],
                                                 gx_rz[d][:, 8 * i + 4 * g:8 * i + 4 * g + 4],
                                                 start=True, stop=False)
                            nc.tensor.matmul(out, w, zh_ap,
                                             start=(g == 2), stop=(t2_ap is None))
                            if t2_ap is not None:
                                nc.tensor.matmul(out, wn, t2_ap,
                                                 start=False, stop=True)

                    # scan position s=0 regions: h_{-1}=0, so Whh@h is just h0
                    produce(0, 0, 0, h0[:, 0:4], None)
                    produce(1, 0, T - 1, h0[:, 4:8], None)

                    def step(s):
                        for d in range(2):
                            i = s if d == 0 else (T - 1) - s
                            rzp, ro, npx, no = region(d, s)
                            rz = sp.tile([H, 8], F32, tag=f"rzs{d}", name=f"rzs{d}")
                            nc.scalar.activation(rz[:], rzp[:, ro:ro + 8], AF.Sigmoid)
                            u = sp.tile([H, 4], F32, tag=f"u{d}", name=f"u{d}")
                            nc.vector.scalar_tensor_tensor(
                                u[:], npx[:, no:no + 4],
                                bhn_t[:, lidx * 2 + d:lidx * 2 + d + 1],
                                rz[:, 0:4], AluOpType.add, AluOpType.mult)
                            v = sp.tile([H, 4], F32, tag=f"v{d}", name=f"v{d}")
                            nc.vector.tensor_tensor(v[:], u[:],
                                                    gx_n[d][:, 4 * i:4 * i + 4],
                                                    AluOpType.add)
                            nt = sp.tile([H, 4], F32, tag=f"nt{d}", name=f"nt{d}")
                            nc.scalar.activation(nt[:], v[:], AF.Tanh)
                            if s == 0:
                                h_prev = h0[:, 4 * d:4 * d + 4]
                            else:
                                prev = (i - 1) if d == 0 else (i + 1)
                                h_prev = mout[d][:, 4 * prev:4 * prev + 4].bitcast(F32)
                            zh = sp.tile([H, 4], F32, tag=f"zh{d}", name=f"zh{d}")
                            nc.vector.tensor_mul(zh[:], rz[:, 4:8], h_prev)
                            t2 = sp.tile([H, 4], F32, tag=f"t2{d}", name=f"t2{d}")
                            nc.vector.scalar_tensor_tensor(
                                t2[:], rz[:, 4:8], -1.0, nt[:],
                                AluOpType.add, AluOpType.mult)
                            if s + 1 < T:
                                i1 = (i + 1) if d == 0 else (i - 1)
                                produce(d, s + 1, i1, zh[:], t2[:])
                            nc.vector.tensor_tensor(mout[d][:, 4 * i:4 * i + 4],
                                                    zh[:], t2[:], AluOpType.subtract)

                    for s in range(T):
                        step(s)

            def boundary(msrc, wih, gx_bias_base):
                # gx_{l+1} = Wih @ m_l  (+ biases)
                gx_rz3 = [gx_rz[d][:].rearrange("p (t k) -> p t k", k=8)
                          for d in range(2)]
                gx_n3 = [gx_n[d][:].rearrange("p (t k) -> p t k", k=4)
                         for d in range(2)]
                msrc3 = [msrc[d][:].rearrange("p (t k) -> p t k", k=4)
                         for d in range(2)]
                with tc.tile_pool(name="bnd", bufs=3, space=bass.MemorySpace.PSUM) as bp:
                    for b in range(BC):
                        for g in range(6):
                            d, gate = divmod(g, 3)
                            pg = bp.tile([H, T], F32, tag="pg")
                            for kb in range(2):
                                nc.tensor.matmul(pg[:],
                                                 wih[kb][:, 100 * g:100 * g + 100],
                                                 msrc3[kb][:, :, b],
                                                 start=(kb == 0), stop=(kb == 1))
                            bias_col = gxb[:, gx_bias_base + g:gx_bias_base + g + 1]
                            if gate < 2:
                                out_ap = gx_rz3[d][:, :, b + 4 * gate]
                            else:
                                out_ap = gx_n3[d][:, :, b]
                            nc.vector.tensor_scalar_add(out_ap, pg[:], bias_col)

            scan_layer(0, m1)
            boundary(m1, wih1, 6)
            scan_layer(1, m2)
            boundary(m2, wih2, 12)
            scan_layer(2, m3)

            # ---------------------------------------------------------------
            # Stage 4: heads
            # ---------------------------------------------------------------
            with tc.tile_pool(name="hd", bufs=4) as hd, \
                 tc.tile_pool(name="hdps", bufs=4, space=bass.MemorySpace.PSUM) as hdps:
                m2v = [m2[d][:].rearrange("p (t k) -> p t k", k=4) for d in range(2)]
                m3v = [m3[d][:].rearrange("p (t k) -> p t k", k=4) for d in range(2)]
                for head, (lgd, mv, outd) in enumerate(
                        ((lgS_dram, m2v, ps_dram), (lgE_dram, m3v, pe_dram))):
                    for b in range(BC):
                        lgt = hd.tile([1, T], F32, tag="lgt")
                        nc.sync.dma_start(out=lgt[:], in_=lgd[b:b + 1, :])
                        lp = hdps.tile([1, T], F32, tag="lp")
                        for d in range(2):
                            nc.tensor.matmul(lp[:],
                                             (wpT[:, 10 * head + 8 + d:10 * head + 9 + d]),
                                             (mv[d][:, :, b].bitcast(F32)),
                                             start=(d == 0), stop=(d == 1))
                        lt = hd.tile([1, T], F32, tag="lt")
                        nc.vector.tensor_tensor(lt[:], lp[:], lgt[:],
                                                AluOpType.add)
                        nmx = hd.tile([1, 1], F32, tag="nmx")
                        nc.vector.tensor_reduce(nmx[:], lt[:], AX.X, AluOpType.max,
                                                negate=True)
                        ex = hd.tile([1, T], F32, tag="ex")
                        sm = hd.tile([1, 1], F32, tag="sm")
                        nc.scalar.activation(ex[:], lt[:], AF.Exp, bias=nmx[:],
                                             accum_out=sm[:])
                        rp = hd.tile([1, 1], F32, tag="rp")
                        nc.vector.reciprocal(rp[:], sm[:])
                        pr = hd.tile([1, T], F32, tag="pr")
                        nc.vector.tensor_scalar_mul(pr[:], ex[:], rp[:])
                        nc.sync.dma_start(out=outd[b:b + 1, :], in_=pr[:])

    nc.compile()
    return nc


def prep_params(inputs, T=T_FULL):
    """Host-side packing of the (tiny) parameter tensors into device layouts."""
    f32 = np.float32
    w_s = inputs["w_s"].astype(f32)
    out = {}

    # whhT: (100, 1800): 3 layers x 2 dirs x 3 gates, each Whh_g^T (100,100)
    whhT = np.zeros((H, 1800), f32)
    bhn = np.zeros((H, 6), f32)
    gxb = np.zeros((H, 18), f32)
    layers = [("mod_Whh0", "mod_bih0", "mod_bhh0"),
              ("mod_Whh1", "mod_bih1", "mod_bhh1"),
              ("out_Whh", "out_bih", "out_bhh")]
    for l, (wk, bik, bhk) in enumerate(layers):
        Whh = inputs[wk].astype(f32)      # (2, 300, 100)
        bih = inputs[bik].astype(f32)     # (2, 300)
        bhh = inputs[bhk].astype(f32)
        for d in range(2):
            for g in range(3):
                whhT[:, l * 600 + d * 300 + g * 100:
                     l * 600 + d * 300 + g * 100 + 100] = \
                    Whh[d, g * 100:(g + 1) * 100, :].T
            bhn[:, l * 2 + d] = bhh[d, 200:300]
            # gx biases in stage-drain order g = d*3 + gate
            for gate in range(3):
                col = l * 6 + d * 3 + gate
                bb = bih[d, gate * 100:(gate + 1) * 100].copy()
                if gate < 2:
                    bb += bhh[d, gate * 100:(gate + 1) * 100]
                gxb[:, col] = bb
    out["whhT"] = whhT
    out["whhTn"] = -whhT
    out["bhn"] = bhn
    out["gxb"] = gxb

    # wih0T: (800, 600) = Wih0^T with dirs stacked on columns
    Wih0 = inputs["mod_Wih0"].astype(f32)  # (2, 300, 800)
    out["wih0T"] = np.concatenate([Wih0[0].T, Wih0[1].T], axis=1)
    Wih1 = inputs["mod_Wih1"].astype(f32)
    out["wih1T"] = np.concatenate([Wih1[0].T, Wih1[1].T], axis=1)
    Wih2 = inputs["out_Wih"].astype(f32)
    out["wih2T"] = np.concatenate([Wih2[0].T, Wih2[1].T], axis=1)

    # wsT: (100, 6): [w_h c0, w_h c1, w_u c0, w_u c1, w_hu c0, w_hu c1]
    wsT = np.zeros((H, 6), f32)
    wsT[:, 0] = w_s[0:100]
    wsT[:, 1] = w_s[100:200]
    wsT[:, 2] = w_s[200:300]
    wsT[:, 3] = w_s[300:400]
    wsT[:, 4] = w_s[400:500]
    wsT[:, 5] = w_s[500:600]
    out["wsT"] = wsT

    # wpT: (100, 20): for each head 10 chunks [c0,c1,u0,u1,cu0,cu1,ch0,ch1,mf,mb]
    wpT = np.zeros((H, 20), f32)
    for hh, key in enumerate(("w_p_start", "w_p_end")):
        wp = inputs[key].astype(f32)
        for kb in range(10):
            wpT[:, 10 * hh + kb] = wp[100 * kb:100 * kb + 100]
    out["wpT"] = wpT
    return out


def kernel(**inputs):
    T = inputs["ctx_emb_c"].shape[1]
    key = (T,)
    if key not in _prog_cache:
        _prog_cache[key] = build_program(T=T)
    nc = _prog_cache[key]

    params = prep_params(inputs, T=T)
    c = np.ascontiguousarray(inputs["ctx_emb_c"].astype(np.float32))
    q = np.ascontiguousarray(inputs["ctx_emb_q"].astype(np.float32))

    in_maps = []
    for core in range(N_CORES):
        m = dict(params)
        m["c"] = c[core * BC:(core + 1) * BC]
        m["q"] = q[core * BC:(core + 1) * BC]
        in_maps.append(m)

    res = run_bass_kernel_spmd(nc, in_maps, list(range(N_CORES)))
    p_start = np.concatenate([r["p_start"] for r in res.results], axis=0)
    p_end = np.concatenate([r["p_end"] for r in res.results], axis=0)
    return p_start, p_end

